# revision 46
# baseline (speedup 1.0000x reference)
"""CapsuleLayer dynamic-routing kernel for 8 Trainium2 NeuronCores.

Per routing iteration (logits constant along o, so routing state is expL[c,n,b]):
    den[c,b] = sum_n expL                      (softmax denominator partials)
    s[c,o,b] = sum_{n,i} W1[c,(n,i),o] x expL  (matmul, expL folded into y=x*expL)
    v = squash(s/den)  via  g = S/((D^2+S)*sqrt(S+eps*D^2)),  v = s_u*g
    a[c,n,b] = sum_i x * (W2^T v)              (matmul + block-diag i-reduce)
    expL *= exp(a)                             (multiplicative logit update)

Sharding: N=1152 split 8 ways (144 nodes/core). The 128 "big" nodes use an
n-major layout (n on partitions, i in the free axis) so y = x*expL needs only a
stride-0 free-axis broadcast on DVE; the 16-node tail keeps an (n,i)-partition
layout with a small broadcast DMA + block-diag reduce. One 87KB AllReduce per
(iteration, capsule-half) carries s-partials + denominators; all collectives sit
alone on the gpsimd queue so its blocking wait never stalls compute.
"""

import sys

sys.path.insert(0, "/opt/trn_rl_repo")

import numpy as np
import ml_dtypes

import concourse.bass as bass
import concourse.bacc as bacc
import concourse.mybir as mybir
from concourse import bass_utils
from concourse.tile import TileContext

BF16 = mybir.dt.bfloat16
F32 = mybir.dt.float32
F16 = mybir.dt.float16
I32 = mybir.dt.int32
AF = mybir.ActivationFunctionType
ALU = mybir.AluOpType

B, N, CI, C, CO = 256, 1152, 8, 10, 16
NCORES = 8
NLOC = N // NCORES          # 144 route nodes per core
K = NLOC * CI               # 1152 local (n,i) contraction length
NB = 128                    # n-major big-chunk rows
NT = NLOC - NB              # 16 ragged tail nodes
EPS = 1e-8
NITER = 3
CB = C * B                  # 2560
HC = C // 2                 # capsules per half-collective
HB = HC * B                 # 1280
RB = 16 * HC + HC           # 85 blob rows: s partials + denominators
DEN0 = float(N)             # iteration-0 softmax denominator (logits all 0)
MAGIC = 0x5F3759DF


def _build_blockdiag() -> np.ndarray:
    """i-reduce lhsT: cols 0..1023 hold 8 [128,128] blocks (chunk j maps
    (n16,i8) row q -> out partition 16j + q//8); cols 1024..1039 map the tail
    chunk's [128,16] block (out partition q//8)."""
    blk = np.zeros((128, 8 * 128 + 16), np.float32)
    for j in range(8):
        for q in range(128):
            blk[q, 128 * j + 16 * j + q // CI] = 1.0
    for q in range(128):
        blk[q, 1024 + q // CI] = 1.0
    return blk.astype(np.float16)


def _bcast_ap(ap, dim_idx, count):
    dims = [list(d) for d in ap.ap]
    dims.insert(dim_idx, [0, count])
    return bass.AP(tensor=ap.tensor, offset=ap.offset, ap=dims)


def _ap_dims(ap, dims):
    return bass.AP(tensor=ap.tensor, offset=ap.offset, ap=[list(d) for d in dims])


def build_kernel():
    nc = bacc.Bacc("TRN2", target_bir_lowering=False, debug=False,
                   num_devices=NCORES)
    xT3_d = nc.dram_tensor("xT3", [128, CI * B], BF16, kind="ExternalInput")
    xA_d = nc.dram_tensor("xA", [K, B], BF16, kind="ExternalInput")
    xT3h_d = nc.dram_tensor("xT3h", [128, CI * B], F16, kind="ExternalInput")
    x9h_d = nc.dram_tensor("x9h", [128, B], F16, kind="ExternalInput")
    w1b_d = nc.dram_tensor("w1b", [128, C * CI * CO], BF16, kind="ExternalInput")
    w10_d = nc.dram_tensor("w10", [128, CI * C * CO], BF16, kind="ExternalInput")
    w19_d = nc.dram_tensor("w19", [128, C * CO], BF16, kind="ExternalInput")
    w2b_d = nc.dram_tensor("w2b", [16, C * CI * 128], F16, kind="ExternalInput")
    w29_d = nc.dram_tensor("w29", [16, C * 128], F16, kind="ExternalInput")
    blk_d = nc.dram_tensor("blk", [128, 1040], F16, kind="ExternalInput")
    vout_d = nc.dram_tensor("vout", [CO, CB], F32, kind="ExternalOutput")

    with TileContext(nc) as tc:
        _emit(tc, xT3_d.ap(), xA_d.ap(), xT3h_d.ap(), x9h_d.ap(), w1b_d.ap(), w10_d.ap(), w19_d.ap(), w2b_d.ap(), w29_d.ap(),
              blk_d.ap(), vout_d.ap())
    nc.compile()
    return nc


def _emit(tc, xT3_d, xA_d, xT3h_d, x9h_d, w1b_d, w10_d, w19_d, w2b_d, w29_d, blk_d, vout_d):
    from contextlib import ExitStack
    with ExitStack() as ctx:
        _emit_body(ctx, tc, xT3_d, xA_d, xT3h_d, x9h_d, w1b_d, w10_d, w19_d, w2b_d, w29_d, blk_d, vout_d)


WARM0 = 64                  # keep-warm filler matmuls after iter0 s-chains
WARMQ = 4                   # keep-warm fillers at each squash boundary
WARMC = 1                   # keep-warm filler after each capsule's s-matmuls
GPS_TREE = False            # run i-reduce tree level 1 on gpsimd


def _emit_body(ctx, tc, xT3_d, xA_d, xT3h_d, x9h_d, w1b_d, w10_d, w19_d, w2b_d, w29_d, blk_d, vout_d):
    nc = tc.nc
    state = ctx.enter_context(tc.tile_pool(name="state", bufs=1))
    z_p = ctx.enter_context(tc.tile_pool(name="zp", bufs=2))
    g_p = ctx.enter_context(tc.tile_pool(name="gp", bufs=2))
    er_p = ctx.enter_context(tc.tile_pool(name="erp", bufs=2))
    ea_p = ctx.enter_context(tc.tile_pool(name="eap", bufs=3))
    dram = ctx.enter_context(tc.tile_pool(name="dram", bufs=2, space="DRAM"))
    ups_p = ctx.enter_context(tc.tile_pool(name="ups", bufs=3, space="PSUM"))
    sps_p = ctx.enter_context(tc.tile_pool(name="sps", bufs=2, space="PSUM"))
    sq_p = ctx.enter_context(tc.tile_pool(name="sqps", bufs=2, space="PSUM"))
    warm_p = ctx.enter_context(tc.tile_pool(name="warmp", bufs=1, space="PSUM"))

    # ---- persistent SBUF state ----
    xT3 = state.tile([128, CI * B], BF16)       # big chunks: [n, (i,b)]
    xA = state.tile([128, 9 * B], BF16)         # (n,i)-chunks: [(n16,i8),(j,b)]
    xT3h = state.tile([128, CI * B], F16)       # f16 n-major x for z-mult
    x9h = state.tile([128, B], F16)             # f16 tail x for z9-mult
    w1b = state.tile([128, C * CI * CO], BF16)  # s lhsT big: [n, (c,i,o)]
    w10 = state.tile([128, CI * C * CO], BF16)  # iter0 lhsT: [n, (i,c,o)]
    w19 = state.tile([128, C * CO], BF16)       # s lhsT tail: [(n,i), (c,o)]
    w2b = state.tile([16, C * CI * 128], F16)   # u lhsT big: [o, (c,i,n)]
    w29 = state.tile([16, C * 128], F16)        # u lhsT tail: [o, (c,(n,i))]
    blk = state.tile([128, 1040], F16)          # i-reduce lhsT blocks
    ones128 = state.tile([128, 1], BF16)
    ones16 = state.tile([16, 1], BF16)
    expL = state.tile([128, CB], BF16)          # softmax numerators [n, (c,b)]
    expL9 = state.tile([16, CB], BF16)          # tail [n16, (c,b)]
    y_all = state.tile([128, C * CI * B], BF16)  # y = x*expL, [n, (c,i,b)]
    y9_all = state.tile([128, CB], BF16)         # tail y, [(n,i), (c,b)]
    s_u = state.tile([16, CB], F32)             # AllReduced s [o, (c,b)]
    s2 = state.tile([16, CB], BF16)
    vb = state.tile([16, CB], F16)              # squashed v (u-matmul rhs)
    v_f = state.tile([16, CB], F32)             # final-iteration v (output)
    ssq_r = state.tile([1, CB], F32)
    den_g = state.tile([128, 20], F32)
    ssq_g = state.tile([128, 20], F32)

    # iter0-critical tensors first, spread across queues so the first
    # s-chains can start as soon as possible
    nc.sync.dma_start(out=w10[:], in_=w10_d[:, :])
    nc.scalar.dma_start(out=xT3[:], in_=xT3_d[:, :])
    nc.gpsimd.dma_start(out=xA[:].rearrange("p (j b) -> p j b", j=9),
                        in_=xA_d.rearrange("(j p) b -> p j b", j=9))
    nc.gpsimd.dma_start(out=w19[:], in_=w19_d[:, :])
    nc.sync.dma_start(out=xT3h[:], in_=xT3h_d[:, :])
    nc.sync.dma_start(out=x9h[:], in_=x9h_d[:, :])
    nc.sync.dma_start(out=w1b[:], in_=w1b_d[:, :])
    nc.scalar.dma_start(out=w2b[:], in_=w2b_d[:, :])
    nc.scalar.dma_start(out=w29[:], in_=w29_d[:, :])
    nc.scalar.dma_start(out=blk[:], in_=blk_d[:, :])
    nc.vector.memset(ones128[:], 1.0)
    nc.vector.memset(ones16[:], 1.0)
    nc.vector.memset(expL[:], 1.0)
    nc.vector.memset(expL9[:], 1.0)

    # blob row maps: r=0 one AR [160,B] rows=(c,o); r=1,2 two ARs each
    # [85,B] rows=(c,o)+den
    blobs = {}
    for r, h, rows in ((0, 0, 16 * C), (1, 0, RB), (1, 1, RB),
                       (2, 0, RB), (2, 1, RB)):
        blobs[(r, h, "in")] = dram.tile(
            [rows, B], F32, tag=f"bi{r}{h}", name=f"blob_in_{r}_{h}")
        blobs[(r, h, "out")] = dram.tile(
            [rows, B], F32, tag=f"bo{r}{h}", name=f"blob_out_{r}_{h}")

    warm = warm_p.tile([1, 512], F32, name="warm")

    def pe_warm(n):
        """Filler matmuls (no data deps beyond xT3) that keep the PE busy
        through a known stall so the HAM clock gate stays at 2.4 GHz."""
        for _ in range(n):
            nc.tensor.matmul(warm[0:1, :], ones128[:], xT3[:, 0:512],
                             start=True, stop=True)

    def collective(r, h):
        """AllReduce + post-AR gathers on the gpsimd queue: it is blocked on
        the AR anyway, so gathers fire the instant the collective completes.
        Gather dst APs are plain slices (rearrange only on the src side)."""
        nc.gpsimd.collective_compute(
            "AllReduce", ALU.add,
            replica_groups=[list(range(NCORES))],
            ins=[blobs[(r, h, "in")].opt()],
            outs=[blobs[(r, h, "out")].opt()],
        )
        blob_out = blobs[(r, h, "out")]
        if r == 0:
            nc.gpsimd.dma_start(
                out=s_u[:, :],
                in_=blob_out[0:16 * C, :].rearrange("(c o) b -> o c b", c=C))
            return
        c0 = h * HC
        hb = slice(c0 * B, (c0 + HC) * B)
        hg = slice(10 * h, 10 * h + 10)
        nc.gpsimd.dma_start(
            out=s_u[:, hb],
            in_=blob_out[0:16 * HC, :].rearrange("(c o) b -> o c b", c=HC))
        src = _ap_dims(blob_out[16 * HC:RB, :], [[10, 128], [1, 10]])
        nc.gpsimd.dma_start(out=den_g[:, hg], in_=src)

    def s_iter0_half(h):
        """Batched priors-sum for capsules [5h,5h+5) in M=32/32/16 chains."""
        c0 = h * HC
        sps = sps_p.tile([16 * HC, B], F32, tag="sps", name=f"sps0_{h}")
        for gc, ncap in ((0, 2), (2, 2), (4, 1)):
            po = 16 * gc
            for i in range(CI):
                lo = i * C * CO + (c0 + gc) * CO
                nc.tensor.matmul(sps[po:po + 16 * ncap, :],
                                 w10[:, lo:lo + ncap * CO],
                                 xT3[:, i * B:(i + 1) * B],
                                 start=(i == 0), stop=False)
            lo9 = (c0 + gc) * CO
            nc.tensor.matmul(sps[po:po + 16 * ncap, :],
                             w19[:, lo9:lo9 + ncap * CO],
                             xA[:, 8 * B:9 * B], start=False, stop=True)
        stage = er_p.tile([16 * HC, B], F32, tag="sstage", name=f"sst0_{h}")
        nc.scalar.copy(stage[:], sps[:])
        nc.sync.dma_start(out=blobs[(0, 0, "in")][80 * h:80 * h + 80, :],
                          in_=stage[:])

    def s_caps(r, stage, c, cc):
        """s-matmuls for capsule c of iteration r, staged for the AR blob.
        Called per-capsule from inside the agreement loop so the PE fills
        its DVE-wait gaps with next-round s work."""
        s_ps = sps_p.tile([16, B], F32, tag="sps", name=f"sps{r}_{c}")
        for i in range(CI):
            lo = (c * CI + i) * CO
            nc.tensor.matmul(s_ps[:], w1b[:, lo:lo + CO],
                             y_all[:, (c * CI + i) * B:(c * CI + i + 1) * B],
                             start=(i == 0), stop=False)
        nc.tensor.matmul(s_ps[:], w19[:, c * CO:(c + 1) * CO],
                         y9_all[:, c * B:(c + 1) * B],
                         start=False, stop=True)
        nc.scalar.copy(stage[:, cc * B:(cc + 1) * B], s_ps[:])

    def s_finish(r, h, stage):
        """den matmuls + blob staging DMAs for iteration r, half h."""
        c0 = h * HC
        den_st = er_p.tile([1, HB], F32, tag="dstage", name=f"dst{r}_{h}")
        for t in range(3):
            w = 512 if t < 2 else 256
            lo = c0 * B + 512 * t
            den_ps = sq_p.tile([1, 512], F32, tag="sq", name=f"den{r}_{h}_{t}")
            nc.tensor.matmul(den_ps[0:1, 0:w], ones128[:],
                             expL[:, lo:lo + w], start=True, stop=False)
            nc.tensor.matmul(den_ps[0:1, 0:w], ones16[:],
                             expL9[:, lo:lo + w], start=False, stop=True)
            nc.scalar.copy(den_st[0:1, 512 * t:512 * t + w], den_ps[0:1, 0:w])
        nc.sync.dma_start(
            out=blobs[(r, h, "in")][0:16 * HC, :].rearrange(
                "(c o) b -> o c b", c=HC),
            in_=stage[:].rearrange("o (c b) -> o c b", c=HC))
        nc.sync.dma_start(out=blobs[(r, h, "in")][16 * HC:RB, :],
                          in_=den_st[:])

    SQW = (512, 512, 256)
    SQO = (0, 512, 1024)

    def squash_half(r, h):
        """v[:, half] = squash(s/den) for capsules [5h, 5h+5).

        The blob gathers ran on gpsimd right after the AR; s^2 runs on the
        scalar engine; g = S * recip((D^2+S) * sqrt(S+eps*D^2)) via scalar
        Sqrt + one fast DVE reciprocal (vs. the old 10-op Newton chain)."""
        c0 = h * HC
        hb = slice(c0 * B, (c0 + HC) * B)
        hg = slice(10 * h, 10 * h + 10)
        pe_warm(WARMQ)

        # ssq = sum_o s_u^2 via ones-matmuls, reshaped to [128,10]
        for t in range(3):
            w, off = SQW[t], SQO[t]
            nc.scalar.square(s2[:, c0 * B + off:c0 * B + off + w],
                             s_u[:, c0 * B + off:c0 * B + off + w])
            ssq_ps = sq_p.tile([1, 512], F32, tag="sq", name=f"ssq_{r}_{h}_{t}")
            nc.tensor.matmul(ssq_ps[0:1, 0:w], ones16[:],
                             s2[:, c0 * B + off:c0 * B + off + w],
                             start=True, stop=True)
            nc.scalar.copy(ssq_r[0:1, c0 * B + off:c0 * B + off + w],
                           ssq_ps[0:1, 0:w])
        nc.scalar.dma_start(out=ssq_g[:, hg], in_=ssq_r[0:1, hb])

        # g = S/((D^2+S)*sqrt(S+eps*D^2)) on [128,10]
        eng = nc.vector
        m1 = g_p.tile([128, 10], F32, tag="g0", name=f"g0_{r}_{h}")
        g = g_p.tile([128, 10], F32, tag="g3", name=f"g3_{r}_{h}")
        q = g_p.tile([128, 10], F32, tag="g4", name=f"g4_{r}_{h}")
        tt = g_p.tile([128, 20], F32, tag="g5", name=f"g5_{r}_{h}")
        d2 = g_p.tile([128, 10], F32, tag="g6", name=f"g6_{r}_{h}")
        S = ssq_g[:, hg]
        if r == 0:
            eng.tensor_scalar(tt[:, 0:10], S, DEN0 * DEN0, None, ALU.add)
            eng.tensor_scalar(tt[:, 10:20], S, EPS * DEN0 * DEN0, None, ALU.add)
        else:
            D = den_g[:, hg]
            eng.tensor_tensor(m1[:], D, D, ALU.mult)
            eng.tensor_tensor(tt[:, 0:10], m1[:], S, ALU.add)
            eng.scalar_tensor_tensor(tt[:, 10:20], m1[:], EPS, S,
                                     ALU.mult, ALU.add)
        rec = g_p.tile([128, 10], F32, tag="g7", name=f"g7_{r}_{h}")
        nc.scalar.sqrt(q[:], tt[:, 10:20])
        eng.tensor_tensor(d2[:], tt[:, 0:10], q[:], ALU.mult)
        eng.reciprocal_approx_fast(rec[:], d2[:])
        eng.tensor_tensor(g[:], S, rec[:], ALU.mult)

        g_stage = dram.tile([HB], F32, tag="gst", name=f"gst_{r}_{h}")
        nc.sync.dma_start(out=_ap_dims(g_stage, [[10, 128], [1, 10]]),
                          in_=g[:])
        g_rep = er_p.tile([16, HB], F32, tag="grep", name=f"grep_{r}_{h}")
        nc.sync.dma_start(out=g_rep[:],
                          in_=_ap_dims(g_stage, [[0, 16], [1, HB]]))
        vdst = v_f if r == NITER - 1 else vb
        # first two capsules first so their u-matmuls start ~1.5us earlier
        nc.vector.tensor_mul(vdst[:, c0 * B:(c0 + 2) * B],
                             s_u[:, c0 * B:(c0 + 2) * B], g_rep[:, 0:2 * B])
        nc.vector.tensor_mul(vdst[:, (c0 + 2) * B:(c0 + HC) * B],
                             s_u[:, (c0 + 2) * B:(c0 + HC) * B],
                             g_rep[:, 2 * B:])

    def agreement_half(r, h):
        """expL *= exp(sum_i x*(W2^T v)); recompute y; then immediately run
        capsule c's next-round s-matmuls so the PE never idles behind DVE."""
        c0 = h * HC
        stage = er_p.tile([16, HB], F32, tag="sstage", name=f"sst{r}_{h}")
        for pair in ((0, 1), (2, 3), (4,)):
            z9p = z_p.tile([128, len(pair) * B], F16, tag="z9p",
                           name=f"z9p_{r}_{h}_{pair[0]}")
            for pc, cc in enumerate(pair):
                c = c0 + cc
                z = z_p.tile([128, CI * B], F16, tag="z", name=f"z_{r}_{c}")
                for q in range(4):
                    u_ps = ups_p.tile([128, 2 * B], F32, tag="ups",
                                      name=f"u_{r}_{c}_{q}")
                    for ii in range(2):
                        i = 2 * q + ii
                        lo = (c * CI + i) * 128
                        nc.tensor.matmul(u_ps[:, ii * B:(ii + 1) * B],
                                         w2b[:, lo:lo + 128],
                                         vb[:, c * B:(c + 1) * B],
                                         start=True, stop=True)
                    if q % 2:
                        # alternate the PSUM drain between scalar and DVE:
                        # two engines in parallel free u_ps faster than the
                        # PE refills, so the u-matmul stream stays dense
                        u_sb = z_p.tile([128, 2 * B], F16, tag="usb",
                                        name=f"usb_{r}_{c}_{q}")
                        nc.scalar.copy(u_sb[:], u_ps[:])
                        nc.vector.tensor_mul(z[:, 2 * q * B:(2 * q + 2) * B],
                                             xT3h[:, 2 * q * B:(2 * q + 2) * B],
                                             u_sb[:])
                    else:
                        nc.vector.tensor_mul(z[:, 2 * q * B:(2 * q + 2) * B],
                                             xT3h[:, 2 * q * B:(2 * q + 2) * B],
                                             u_ps[:])
                u9_ps = ups_p.tile([128, 2 * B], F32, tag="ups",
                                   name=f"u9_{r}_{c}")
                nc.tensor.matmul(u9_ps[:, 0:B], w29[:, c * 128:(c + 1) * 128],
                                 vb[:, c * B:(c + 1) * B],
                                 start=True, stop=True)
                nc.vector.tensor_mul(z9p[:, pc * B:(pc + 1) * B], x9h[:],
                                     u9_ps[:, 0:B])
                # i-reduce tree on the free axis
                t1 = z_p.tile([128, 4 * B], F16, tag="t1", name=f"t1_{r}_{c}")
                nc.vector.tensor_add(t1[:], z[:, 0:4 * B], z[:, 4 * B:8 * B])
                t2 = z_p.tile([128, 2 * B], F16, tag="t2", name=f"t2_{r}_{c}")
                nc.vector.tensor_add(t2[:], t1[:, 0:2 * B], t1[:, 2 * B:4 * B])
                av = z_p.tile([128, B], F16, tag="av", name=f"av_{r}_{c}")
                nc.vector.tensor_add(av[:], t2[:, 0:B], t2[:, B:2 * B])
                ea = ea_p.tile([128, B], BF16, tag="ea", name=f"ea_{r}_{c}")
                nc.scalar.activation(ea[:], av[:], AF.Exp)
                nc.vector.tensor_mul(expL[:, c * B:(c + 1) * B],
                                     expL[:, c * B:(c + 1) * B], ea[:])
                eb = _ap_dims(expL[:, c * B:(c + 1) * B],
                              [list(expL[:].ap[0]), [0, CI], [1, B]])
                nc.vector.tensor_mul(
                    y_all[:, c * CI * B:(c + 1) * CI * B].rearrange(
                        "p (i b) -> p i b", i=CI),
                    xT3[:].rearrange("p (i b) -> p i b", i=CI), eb)
            # tail: block-diag i-reduce on PE, one matmul per pair
            w = len(pair) * B
            a9_ps = sq_p.tile([16, 2 * B], F32, tag="sq",
                              name=f"a9_{r}_{h}_{pair[0]}")
            nc.tensor.matmul(a9_ps[:, 0:w], blk[:, 1024:1040], z9p[:],
                             start=True, stop=True)
            ea9 = ea_p.tile([16, 2 * B], BF16, tag="ea9",
                            name=f"ea9_{r}_{h}_{pair[0]}")
            nc.scalar.activation(ea9[:, 0:w], a9_ps[:, 0:w], AF.Exp)
            pl = slice((c0 + pair[0]) * B, (c0 + pair[-1] + 1) * B)
            nc.vector.tensor_mul(expL9[:, pl], expL9[:, pl], ea9[:, 0:w])
            for cc in pair:
                c = c0 + cc
                erep9 = er_p.tile([128, B], BF16, tag="er9",
                                  name=f"er9_{r}_{c}")
                nc.sync.dma_start(
                    out=erep9[:],
                    in_=_bcast_ap(expL9[:, c * B:(c + 1) * B], 1, CI))
                nc.vector.tensor_mul(y9_all[:, c * B:(c + 1) * B],
                                     xA[:, 8 * B:9 * B], erep9[:])
            for cc in pair:
                s_caps(r + 1, stage, c0 + cc, cc)
                pe_warm(WARMC)
        s_finish(r + 1, h, stage)

    # ---- schedule: iter0 uses one merged AR (its pair was serial on the CC
    # stream with nothing to overlap); each later half-AR ping-pongs against
    # the other half's squash+agreement+s compute block. ----
    s_iter0_half(0)
    s_iter0_half(1)
    collective(0, 0)
    pe_warm(WARM0)
    squash_half(0, 0)
    agreement_half(0, 0)      # also emits s(1,0) + its blob staging
    collective(1, 0)
    squash_half(0, 1)
    agreement_half(0, 1)
    collective(1, 1)
    squash_half(1, 0)
    agreement_half(1, 0)
    collective(2, 0)
    squash_half(1, 1)
    agreement_half(1, 1)
    collective(2, 1)
    squash_half(2, 0)
    nc.sync.dma_start(out=vout_d[:, 0:HB], in_=v_f[:, 0:HB])
    squash_half(2, 1)
    nc.sync.dma_start(out=vout_d[:, HB:CB], in_=v_f[:, HB:CB])


def _prep_inputs(x: np.ndarray, route_weights: np.ndarray):
    """Host-side sharding + layout prep. Returns per-core input maps."""
    bf = ml_dtypes.bfloat16
    blk = _build_blockdiag()
    in_maps = []
    for k in range(NCORES):
        n0 = k * NLOC
        xb = x[:, n0:n0 + NB, :]                  # [B, 128, 8]
        wb = route_weights[:, n0:n0 + NB]         # [C, 128, 8, 16]
        wt = route_weights[:, n0 + NB:n0 + NLOC]  # [C, 16, 8, 16]
        xT3 = np.ascontiguousarray(
            xb.transpose(1, 2, 0).reshape(128, CI * B)).astype(bf)
        xAf = np.ascontiguousarray(
            x[:, n0:n0 + NLOC, :].transpose(1, 2, 0).reshape(K, B))
        xA = xAf.astype(bf)
        xT3h = np.ascontiguousarray(
            xb.transpose(1, 2, 0).reshape(128, CI * B)).astype(np.float16)
        x9h = xAf[8 * 128:, :].astype(np.float16)
        w1b = np.ascontiguousarray(
            wb.transpose(1, 0, 2, 3).reshape(128, C * CI * CO)).astype(bf)
        w10 = np.ascontiguousarray(
            wb.transpose(1, 2, 0, 3).reshape(128, CI * C * CO)).astype(bf)
        w19 = np.ascontiguousarray(
            wt.reshape(C, NT * CI, CO).transpose(1, 0, 2).reshape(
                128, C * CO)).astype(bf)
        w2b = np.ascontiguousarray(
            wb.transpose(3, 0, 2, 1).reshape(CO, C * CI * 128)).astype(
                np.float16)
        wt = route_weights[:, n0 + NB:n0 + NLOC]
        w29 = np.ascontiguousarray(
            wt.reshape(C, NT * CI, CO).transpose(2, 0, 1).reshape(
                CO, C * 128)).astype(np.float16)
        in_maps.append({"xT3": xT3, "xA": xA, "xT3h": xT3h, "x9h": x9h,
                        "w1b": w1b, "w10": w10, "w19": w19, "w2b": w2b,
                        "w29": w29, "blk": blk})
    return in_maps


_NC_CACHE = {}


def _get_nc():
    if "nc" not in _NC_CACHE:
        _NC_CACHE["nc"] = build_kernel()
    return _NC_CACHE["nc"]


def _postprocess(v: np.ndarray) -> np.ndarray:
    out = v.reshape(CO, C, B).transpose(1, 2, 0)[:, :, None, None, :]
    return np.ascontiguousarray(out.astype(np.float32))


def kernel(x: np.ndarray, route_weights: np.ndarray) -> np.ndarray:
    nc = _get_nc()
    in_maps = _prep_inputs(np.asarray(x, np.float32),
                           np.asarray(route_weights, np.float32))
    res = bass_utils.run_bass_kernel_spmd(nc, in_maps,
                                          core_ids=list(range(NCORES)))
    return _postprocess(np.asarray(res.results[0]["vout"], np.float32))


def kernel_sim(x: np.ndarray, route_weights: np.ndarray) -> np.ndarray:
    """CoreSim (multi-core simulator) path for correctness debugging."""
    from concourse.bass_interp import MultiCoreSim
    nc = _get_nc()
    in_maps = _prep_inputs(np.asarray(x, np.float32),
                           np.asarray(route_weights, np.float32))
    sim = MultiCoreSim(nc, num_cores=NCORES)
    for i, core in sim.cores.items():
        for name, arr in in_maps[i].items():
            core.tensor(name)[:] = arr
    sim.simulate(check_with_hw=False)
    return _postprocess(np.asarray(sim.cores[0].tensor("vout"), np.float32))



# revision 47
# speedup vs baseline: 1.0805x; 1.0805x over previous
"""CapsuleLayer dynamic-routing kernel for 8 Trainium2 NeuronCores.

Per routing iteration (logits constant along o, so routing state is expL[c,n,b]):
    den[c,b] = sum_n expL                      (softmax denominator partials)
    s[c,o,b] = sum_{n,i} W1[c,(n,i),o] x expL  (matmul, expL folded into y=x*expL)
    v = squash(s/den)  via  g = S/((D^2+S)*sqrt(S+eps*D^2)),  v = s_u*g
    a[c,n,b] = sum_i x * (W2^T v)              (matmul + block-diag i-reduce)
    expL *= exp(a)                             (multiplicative logit update)

Sharding: N=1152 split 8 ways (144 nodes/core). The 128 "big" nodes use an
n-major layout (n on partitions, i in the free axis) so y = x*expL needs only a
stride-0 free-axis broadcast on DVE; the 16-node tail keeps an (n,i)-partition
layout with a small broadcast DMA + block-diag reduce. One 87KB AllReduce per
(iteration, capsule-half) carries s-partials + denominators; all collectives sit
alone on the gpsimd queue so its blocking wait never stalls compute.
"""

import sys

sys.path.insert(0, "/opt/trn_rl_repo")

import numpy as np
import ml_dtypes

import concourse.bass as bass
import concourse.bacc as bacc
import concourse.mybir as mybir
from concourse import bass_utils
from concourse.tile import TileContext

BF16 = mybir.dt.bfloat16
F32 = mybir.dt.float32
F16 = mybir.dt.float16
I32 = mybir.dt.int32
AF = mybir.ActivationFunctionType
ALU = mybir.AluOpType

B, N, CI, C, CO = 256, 1152, 8, 10, 16
NCORES = 8
NLOC = N // NCORES          # 144 route nodes per core
K = NLOC * CI               # 1152 local (n,i) contraction length
NB = 128                    # n-major big-chunk rows
NT = NLOC - NB              # 16 ragged tail nodes
EPS = 1e-8
NITER = 3
CB = C * B                  # 2560
HC = C // 2                 # capsules per half-collective
HB = HC * B                 # 1280
RB = 16 * HC + HC           # 85 blob rows: s partials + denominators
DEN0 = float(N)             # iteration-0 softmax denominator (logits all 0)
MAGIC = 0x5F3759DF


def _build_blockdiag() -> np.ndarray:
    """i-reduce lhsT: cols 0..1023 hold 8 [128,128] blocks (chunk j maps
    (n16,i8) row q -> out partition 16j + q//8); cols 1024..1039 map the tail
    chunk's [128,16] block (out partition q//8)."""
    blk = np.zeros((128, 8 * 128 + 16), np.float32)
    for j in range(8):
        for q in range(128):
            blk[q, 128 * j + 16 * j + q // CI] = 1.0
    for q in range(128):
        blk[q, 1024 + q // CI] = 1.0
    return blk.astype(np.float16)


def _bcast_ap(ap, dim_idx, count):
    dims = [list(d) for d in ap.ap]
    dims.insert(dim_idx, [0, count])
    return bass.AP(tensor=ap.tensor, offset=ap.offset, ap=dims)


def _ap_dims(ap, dims):
    return bass.AP(tensor=ap.tensor, offset=ap.offset, ap=[list(d) for d in dims])


def build_kernel():
    nc = bacc.Bacc("TRN2", target_bir_lowering=False, debug=False,
                   num_devices=NCORES)
    xT3_d = nc.dram_tensor("xT3", [128, CI * B], BF16, kind="ExternalInput")
    xA_d = nc.dram_tensor("xA", [K, B], BF16, kind="ExternalInput")
    xT3h_d = nc.dram_tensor("xT3h", [128, CI * B], F16, kind="ExternalInput")
    x9h_d = nc.dram_tensor("x9h", [128, B], F16, kind="ExternalInput")
    w1b_d = nc.dram_tensor("w1b", [128, C * CI * CO], BF16, kind="ExternalInput")
    w10_d = nc.dram_tensor("w10", [128, CI * C * CO], BF16, kind="ExternalInput")
    w19_d = nc.dram_tensor("w19", [128, C * CO], BF16, kind="ExternalInput")
    w2b_d = nc.dram_tensor("w2b", [16, C * CI * 128], F16, kind="ExternalInput")
    w29_d = nc.dram_tensor("w29", [16, C * 128], F16, kind="ExternalInput")
    blk_d = nc.dram_tensor("blk", [128, 1040], F16, kind="ExternalInput")
    vout_d = nc.dram_tensor("vout", [CO, CB], F32, kind="ExternalOutput")

    with TileContext(nc) as tc:
        _emit(tc, xT3_d.ap(), xA_d.ap(), xT3h_d.ap(), x9h_d.ap(), w1b_d.ap(), w10_d.ap(), w19_d.ap(), w2b_d.ap(), w29_d.ap(),
              blk_d.ap(), vout_d.ap())
    nc.compile()
    return nc


def _emit(tc, xT3_d, xA_d, xT3h_d, x9h_d, w1b_d, w10_d, w19_d, w2b_d, w29_d, blk_d, vout_d):
    from contextlib import ExitStack
    with ExitStack() as ctx:
        _emit_body(ctx, tc, xT3_d, xA_d, xT3h_d, x9h_d, w1b_d, w10_d, w19_d, w2b_d, w29_d, blk_d, vout_d)


WARM0 = 64                  # keep-warm filler matmuls after iter0 s-chains
WARMQ = 4                   # keep-warm fillers at each squash boundary
GPS_TREE = False            # run i-reduce tree level 1 on gpsimd


def _emit_body(ctx, tc, xT3_d, xA_d, xT3h_d, x9h_d, w1b_d, w10_d, w19_d, w2b_d, w29_d, blk_d, vout_d):
    nc = tc.nc
    state = ctx.enter_context(tc.tile_pool(name="state", bufs=1))
    z_p = ctx.enter_context(tc.tile_pool(name="zp", bufs=2))
    g_p = ctx.enter_context(tc.tile_pool(name="gp", bufs=2))
    er_p = ctx.enter_context(tc.tile_pool(name="erp", bufs=2))
    ea_p = ctx.enter_context(tc.tile_pool(name="eap", bufs=3))
    dram = ctx.enter_context(tc.tile_pool(name="dram", bufs=2, space="DRAM"))
    ups_p = ctx.enter_context(tc.tile_pool(name="ups", bufs=3, space="PSUM"))
    sps_p = ctx.enter_context(tc.tile_pool(name="sps", bufs=2, space="PSUM"))
    sq_p = ctx.enter_context(tc.tile_pool(name="sqps", bufs=2, space="PSUM"))
    warm_p = ctx.enter_context(tc.tile_pool(name="warmp", bufs=1, space="PSUM"))

    # ---- persistent SBUF state ----
    xT3 = state.tile([128, CI * B], BF16)       # big chunks: [n, (i,b)]
    xA = state.tile([128, 9 * B], BF16)         # (n,i)-chunks: [(n16,i8),(j,b)]
    xT3h = state.tile([128, CI * B], F16)       # f16 n-major x for z-mult
    x9h = state.tile([128, B], F16)             # f16 tail x for z9-mult
    w1b = state.tile([128, C * CI * CO], BF16)  # s lhsT big: [n, (c,i,o)]
    w10 = state.tile([128, CI * C * CO], BF16)  # iter0 lhsT: [n, (i,c,o)]
    w19 = state.tile([128, C * CO], BF16)       # s lhsT tail: [(n,i), (c,o)]
    w2b = state.tile([16, C * CI * 128], F16)   # u lhsT big: [o, (c,i,n)]
    w29 = state.tile([16, C * 128], F16)        # u lhsT tail: [o, (c,(n,i))]
    blk = state.tile([128, 1040], F16)          # i-reduce lhsT blocks
    ones128 = state.tile([128, 1], BF16)
    ones16 = state.tile([16, 1], BF16)
    expL = state.tile([128, CB], BF16)          # softmax numerators [n, (c,b)]
    expL9 = state.tile([16, CB], BF16)          # tail [n16, (c,b)]
    y_all = state.tile([128, C * CI * B], BF16)  # y = x*expL, [n, (c,i,b)]
    y9_all = state.tile([128, CB], BF16)         # tail y, [(n,i), (c,b)]
    s_u = state.tile([16, CB], F32)             # AllReduced s [o, (c,b)]
    s2 = state.tile([16, CB], BF16)
    vb = state.tile([16, CB], F16)              # squashed v (u-matmul rhs)
    v_f = state.tile([16, CB], F32)             # final-iteration v (output)
    ssq_r = state.tile([1, CB], F32)
    den_g = state.tile([128, 20], F32)
    ssq_g = state.tile([128, 20], F32)

    # iter0-critical tensors first, spread across queues so the first
    # s-chains can start as soon as possible
    nc.sync.dma_start(out=w10[:], in_=w10_d[:, :])
    nc.scalar.dma_start(out=xT3[:], in_=xT3_d[:, :])
    nc.gpsimd.dma_start(out=xA[:].rearrange("p (j b) -> p j b", j=9),
                        in_=xA_d.rearrange("(j p) b -> p j b", j=9))
    nc.gpsimd.dma_start(out=w19[:], in_=w19_d[:, :])
    nc.sync.dma_start(out=xT3h[:], in_=xT3h_d[:, :])
    nc.sync.dma_start(out=x9h[:], in_=x9h_d[:, :])
    nc.sync.dma_start(out=w1b[:], in_=w1b_d[:, :])
    nc.scalar.dma_start(out=w2b[:], in_=w2b_d[:, :])
    nc.scalar.dma_start(out=w29[:], in_=w29_d[:, :])
    nc.scalar.dma_start(out=blk[:], in_=blk_d[:, :])
    nc.vector.memset(ones128[:], 1.0)
    nc.vector.memset(ones16[:], 1.0)
    nc.vector.memset(expL[:], 1.0)
    nc.vector.memset(expL9[:], 1.0)

    # blob row maps: r=0 one AR [160,B] rows=(c,o); r=1,2 two ARs each
    # [85,B] rows=(c,o)+den
    blobs = {}
    for r, h, rows in ((0, 0, 16 * C), (1, 0, RB), (1, 1, RB),
                       (2, 0, RB), (2, 1, RB)):
        blobs[(r, h, "in")] = dram.tile(
            [rows, B], F32, tag=f"bi{r}{h}", name=f"blob_in_{r}_{h}")
        blobs[(r, h, "out")] = dram.tile(
            [rows, B], F32, tag=f"bo{r}{h}", name=f"blob_out_{r}_{h}")

    warm = warm_p.tile([1, 512], F32, name="warm")

    def pe_warm(n):
        """Filler matmuls (no data deps beyond xT3) that keep the PE busy
        through a known stall so the HAM clock gate stays at 2.4 GHz."""
        for _ in range(n):
            nc.tensor.matmul(warm[0:1, :], ones128[:], xT3[:, 0:512],
                             start=True, stop=True)

    def collective(r, h):
        """AllReduce + post-AR gathers on the gpsimd queue: it is blocked on
        the AR anyway, so gathers fire the instant the collective completes.
        Gather dst APs are plain slices (rearrange only on the src side)."""
        nc.gpsimd.collective_compute(
            "AllReduce", ALU.add,
            replica_groups=[list(range(NCORES))],
            ins=[blobs[(r, h, "in")].opt()],
            outs=[blobs[(r, h, "out")].opt()],
        )
        blob_out = blobs[(r, h, "out")]
        if r == 0:
            nc.gpsimd.dma_start(
                out=s_u[:, :],
                in_=blob_out[0:16 * C, :].rearrange("(c o) b -> o c b", c=C))
            return
        c0 = h * HC
        hb = slice(c0 * B, (c0 + HC) * B)
        hg = slice(10 * h, 10 * h + 10)
        nc.gpsimd.dma_start(
            out=s_u[:, hb],
            in_=blob_out[0:16 * HC, :].rearrange("(c o) b -> o c b", c=HC))
        src = _ap_dims(blob_out[16 * HC:RB, :], [[10, 128], [1, 10]])
        nc.gpsimd.dma_start(out=den_g[:, hg], in_=src)

    def s_iter0_half(h):
        """Batched priors-sum for capsules [5h,5h+5) in M=32/32/16 chains."""
        c0 = h * HC
        sps = sps_p.tile([16 * HC, B], F32, tag="sps", name=f"sps0_{h}")
        for gc, ncap in ((0, 2), (2, 2), (4, 1)):
            po = 16 * gc
            for i in range(CI):
                lo = i * C * CO + (c0 + gc) * CO
                nc.tensor.matmul(sps[po:po + 16 * ncap, :],
                                 w10[:, lo:lo + ncap * CO],
                                 xT3[:, i * B:(i + 1) * B],
                                 start=(i == 0), stop=False)
            lo9 = (c0 + gc) * CO
            nc.tensor.matmul(sps[po:po + 16 * ncap, :],
                             w19[:, lo9:lo9 + ncap * CO],
                             xA[:, 8 * B:9 * B], start=False, stop=True)
        stage = er_p.tile([16 * HC, B], F32, tag="sstage", name=f"sst0_{h}")
        nc.scalar.copy(stage[:], sps[:])
        nc.sync.dma_start(out=blobs[(0, 0, "in")][80 * h:80 * h + 80, :],
                          in_=stage[:])

    def s_caps(r, stage, c, cc):
        """s-matmuls for capsule c of iteration r, staged for the AR blob.
        Called per-capsule from inside the agreement loop so the PE fills
        its DVE-wait gaps with next-round s work."""
        s_ps = sps_p.tile([16, B], F32, tag="sps", name=f"sps{r}_{c}")
        for i in range(CI):
            lo = (c * CI + i) * CO
            nc.tensor.matmul(s_ps[:], w1b[:, lo:lo + CO],
                             y_all[:, (c * CI + i) * B:(c * CI + i + 1) * B],
                             start=(i == 0), stop=False)
        nc.tensor.matmul(s_ps[:], w19[:, c * CO:(c + 1) * CO],
                         y9_all[:, c * B:(c + 1) * B],
                         start=False, stop=True)
        nc.scalar.copy(stage[:, cc * B:(cc + 1) * B], s_ps[:])

    def s_finish(r, h, stage):
        """den matmuls + blob staging DMAs for iteration r, half h."""
        c0 = h * HC
        den_st = er_p.tile([1, HB], F32, tag="dstage", name=f"dst{r}_{h}")
        for t in range(3):
            w = 512 if t < 2 else 256
            lo = c0 * B + 512 * t
            den_ps = sq_p.tile([1, 512], F32, tag="sq", name=f"den{r}_{h}_{t}")
            nc.tensor.matmul(den_ps[0:1, 0:w], ones128[:],
                             expL[:, lo:lo + w], start=True, stop=False)
            nc.tensor.matmul(den_ps[0:1, 0:w], ones16[:],
                             expL9[:, lo:lo + w], start=False, stop=True)
            nc.scalar.copy(den_st[0:1, 512 * t:512 * t + w], den_ps[0:1, 0:w])
        nc.sync.dma_start(
            out=blobs[(r, h, "in")][0:16 * HC, :].rearrange(
                "(c o) b -> o c b", c=HC),
            in_=stage[:].rearrange("o (c b) -> o c b", c=HC))
        nc.sync.dma_start(out=blobs[(r, h, "in")][16 * HC:RB, :],
                          in_=den_st[:])

    SQW = (512, 512, 256)
    SQO = (0, 512, 1024)

    def squash_half(r, h):
        """v[:, half] = squash(s/den) for capsules [5h, 5h+5).

        The blob gathers ran on gpsimd right after the AR; s^2 runs on the
        scalar engine; g = S * recip((D^2+S) * sqrt(S+eps*D^2)) via scalar
        Sqrt + one fast DVE reciprocal (vs. the old 10-op Newton chain)."""
        c0 = h * HC
        hb = slice(c0 * B, (c0 + HC) * B)
        hg = slice(10 * h, 10 * h + 10)
        pe_warm(WARMQ)

        # ssq = sum_o s_u^2 via ones-matmuls, reshaped to [128,10]
        for t in range(3):
            w, off = SQW[t], SQO[t]
            nc.scalar.square(s2[:, c0 * B + off:c0 * B + off + w],
                             s_u[:, c0 * B + off:c0 * B + off + w])
            ssq_ps = sq_p.tile([1, 512], F32, tag="sq", name=f"ssq_{r}_{h}_{t}")
            nc.tensor.matmul(ssq_ps[0:1, 0:w], ones16[:],
                             s2[:, c0 * B + off:c0 * B + off + w],
                             start=True, stop=True)
            nc.scalar.copy(ssq_r[0:1, c0 * B + off:c0 * B + off + w],
                           ssq_ps[0:1, 0:w])
        nc.scalar.dma_start(out=ssq_g[:, hg], in_=ssq_r[0:1, hb])

        # g = S/((D^2+S)*sqrt(S+eps*D^2)) on [128,10]
        eng = nc.vector
        m1 = g_p.tile([128, 10], F32, tag="g0", name=f"g0_{r}_{h}")
        g = g_p.tile([128, 10], F32, tag="g3", name=f"g3_{r}_{h}")
        q = g_p.tile([128, 10], F32, tag="g4", name=f"g4_{r}_{h}")
        tt = g_p.tile([128, 20], F32, tag="g5", name=f"g5_{r}_{h}")
        d2 = g_p.tile([128, 10], F32, tag="g6", name=f"g6_{r}_{h}")
        S = ssq_g[:, hg]
        if r == 0:
            eng.tensor_scalar(tt[:, 0:10], S, DEN0 * DEN0, None, ALU.add)
            eng.tensor_scalar(tt[:, 10:20], S, EPS * DEN0 * DEN0, None, ALU.add)
        else:
            D = den_g[:, hg]
            eng.tensor_tensor(m1[:], D, D, ALU.mult)
            eng.tensor_tensor(tt[:, 0:10], m1[:], S, ALU.add)
            eng.scalar_tensor_tensor(tt[:, 10:20], m1[:], EPS, S,
                                     ALU.mult, ALU.add)
        rec = g_p.tile([128, 10], F32, tag="g7", name=f"g7_{r}_{h}")
        nc.scalar.sqrt(q[:], tt[:, 10:20])
        eng.tensor_tensor(d2[:], tt[:, 0:10], q[:], ALU.mult)
        eng.reciprocal_approx_fast(rec[:], d2[:])
        eng.tensor_tensor(g[:], S, rec[:], ALU.mult)

        g_stage = dram.tile([HB], F32, tag="gst", name=f"gst_{r}_{h}")
        nc.sync.dma_start(out=_ap_dims(g_stage, [[10, 128], [1, 10]]),
                          in_=g[:])
        g_rep = er_p.tile([16, HB], F32, tag="grep", name=f"grep_{r}_{h}")
        nc.sync.dma_start(out=g_rep[:],
                          in_=_ap_dims(g_stage, [[0, 16], [1, HB]]))
        vdst = v_f if r == NITER - 1 else vb
        # first two capsules first so their u-matmuls start ~1.5us earlier
        nc.vector.tensor_mul(vdst[:, c0 * B:(c0 + 2) * B],
                             s_u[:, c0 * B:(c0 + 2) * B], g_rep[:, 0:2 * B])
        nc.vector.tensor_mul(vdst[:, (c0 + 2) * B:(c0 + HC) * B],
                             s_u[:, (c0 + 2) * B:(c0 + HC) * B],
                             g_rep[:, 2 * B:])

    def agreement_half(r, h):
        """expL *= exp(sum_i x*(W2^T v)); recompute y; then immediately run
        capsule c's next-round s-matmuls so the PE never idles behind DVE."""
        c0 = h * HC
        stage = er_p.tile([16, HB], F32, tag="sstage", name=f"sst{r}_{h}")
        for pair in ((0, 1), (2, 3), (4,)):
            z9p = z_p.tile([128, len(pair) * B], F16, tag="z9p",
                           name=f"z9p_{r}_{h}_{pair[0]}")
            for pc, cc in enumerate(pair):
                c = c0 + cc
                z = z_p.tile([128, CI * B], F16, tag="z", name=f"z_{r}_{c}")
                for q in range(4):
                    u_ps = ups_p.tile([128, 2 * B], F32, tag="ups",
                                      name=f"u_{r}_{c}_{q}")
                    for ii in range(2):
                        i = 2 * q + ii
                        lo = (c * CI + i) * 128
                        nc.tensor.matmul(u_ps[:, ii * B:(ii + 1) * B],
                                         w2b[:, lo:lo + 128],
                                         vb[:, c * B:(c + 1) * B],
                                         start=True, stop=True)
                    if q % 2:
                        # alternate the PSUM drain between scalar and DVE:
                        # two engines in parallel free u_ps faster than the
                        # PE refills, so the u-matmul stream stays dense
                        u_sb = z_p.tile([128, 2 * B], F16, tag="usb",
                                        name=f"usb_{r}_{c}_{q}")
                        nc.scalar.copy(u_sb[:], u_ps[:])
                        nc.vector.tensor_mul(z[:, 2 * q * B:(2 * q + 2) * B],
                                             xT3h[:, 2 * q * B:(2 * q + 2) * B],
                                             u_sb[:])
                    else:
                        nc.vector.tensor_mul(z[:, 2 * q * B:(2 * q + 2) * B],
                                             xT3h[:, 2 * q * B:(2 * q + 2) * B],
                                             u_ps[:])
                u9_ps = ups_p.tile([128, 2 * B], F32, tag="ups",
                                   name=f"u9_{r}_{c}")
                nc.tensor.matmul(u9_ps[:, 0:B], w29[:, c * 128:(c + 1) * 128],
                                 vb[:, c * B:(c + 1) * B],
                                 start=True, stop=True)
                nc.vector.tensor_mul(z9p[:, pc * B:(pc + 1) * B], x9h[:],
                                     u9_ps[:, 0:B])
                # i-reduce tree on the free axis
                t1 = z_p.tile([128, 4 * B], F16, tag="t1", name=f"t1_{r}_{c}")
                nc.vector.tensor_add(t1[:], z[:, 0:4 * B], z[:, 4 * B:8 * B])
                t2 = z_p.tile([128, 2 * B], F16, tag="t2", name=f"t2_{r}_{c}")
                nc.vector.tensor_add(t2[:], t1[:, 0:2 * B], t1[:, 2 * B:4 * B])
                av = z_p.tile([128, B], F16, tag="av", name=f"av_{r}_{c}")
                nc.vector.tensor_add(av[:], t2[:, 0:B], t2[:, B:2 * B])
                ea = ea_p.tile([128, B], BF16, tag="ea", name=f"ea_{r}_{c}")
                nc.scalar.activation(ea[:], av[:], AF.Exp)
                nc.vector.tensor_mul(expL[:, c * B:(c + 1) * B],
                                     expL[:, c * B:(c + 1) * B], ea[:])
                eb = _ap_dims(expL[:, c * B:(c + 1) * B],
                              [list(expL[:].ap[0]), [0, CI], [1, B]])
                nc.vector.tensor_mul(
                    y_all[:, c * CI * B:(c + 1) * CI * B].rearrange(
                        "p (i b) -> p i b", i=CI),
                    xT3[:].rearrange("p (i b) -> p i b", i=CI), eb)
            # tail: block-diag i-reduce on PE, one matmul per pair
            w = len(pair) * B
            a9_ps = sq_p.tile([16, 2 * B], F32, tag="sq",
                              name=f"a9_{r}_{h}_{pair[0]}")
            nc.tensor.matmul(a9_ps[:, 0:w], blk[:, 1024:1040], z9p[:],
                             start=True, stop=True)
            ea9 = ea_p.tile([16, 2 * B], BF16, tag="ea9",
                            name=f"ea9_{r}_{h}_{pair[0]}")
            nc.scalar.activation(ea9[:, 0:w], a9_ps[:, 0:w], AF.Exp)
            pl = slice((c0 + pair[0]) * B, (c0 + pair[-1] + 1) * B)
            nc.vector.tensor_mul(expL9[:, pl], expL9[:, pl], ea9[:, 0:w])
            for cc in pair:
                c = c0 + cc
                erep9 = er_p.tile([128, B], BF16, tag="er9",
                                  name=f"er9_{r}_{c}")
                nc.sync.dma_start(
                    out=erep9[:],
                    in_=_bcast_ap(expL9[:, c * B:(c + 1) * B], 1, CI))
                nc.vector.tensor_mul(y9_all[:, c * B:(c + 1) * B],
                                     xA[:, 8 * B:9 * B], erep9[:])
            for cc in pair:
                s_caps(r + 1, stage, c0 + cc, cc)
        s_finish(r + 1, h, stage)

    # ---- schedule: iter0 uses one merged AR (its pair was serial on the CC
    # stream with nothing to overlap); each later half-AR ping-pongs against
    # the other half's squash+agreement+s compute block. ----
    s_iter0_half(0)
    s_iter0_half(1)
    collective(0, 0)
    pe_warm(WARM0)
    squash_half(0, 0)
    agreement_half(0, 0)      # also emits s(1,0) + its blob staging
    collective(1, 0)
    squash_half(0, 1)
    agreement_half(0, 1)
    collective(1, 1)
    squash_half(1, 0)
    agreement_half(1, 0)
    collective(2, 0)
    squash_half(1, 1)
    agreement_half(1, 1)
    collective(2, 1)
    squash_half(2, 0)
    nc.sync.dma_start(out=vout_d[:, 0:HB], in_=v_f[:, 0:HB])
    squash_half(2, 1)
    nc.sync.dma_start(out=vout_d[:, HB:CB], in_=v_f[:, HB:CB])


def _prep_inputs(x: np.ndarray, route_weights: np.ndarray):
    """Host-side sharding + layout prep. Returns per-core input maps."""
    bf = ml_dtypes.bfloat16
    blk = _build_blockdiag()
    in_maps = []
    for k in range(NCORES):
        n0 = k * NLOC
        xb = x[:, n0:n0 + NB, :]                  # [B, 128, 8]
        wb = route_weights[:, n0:n0 + NB]         # [C, 128, 8, 16]
        wt = route_weights[:, n0 + NB:n0 + NLOC]  # [C, 16, 8, 16]
        xT3 = np.ascontiguousarray(
            xb.transpose(1, 2, 0).reshape(128, CI * B)).astype(bf)
        xAf = np.ascontiguousarray(
            x[:, n0:n0 + NLOC, :].transpose(1, 2, 0).reshape(K, B))
        xA = xAf.astype(bf)
        xT3h = np.ascontiguousarray(
            xb.transpose(1, 2, 0).reshape(128, CI * B)).astype(np.float16)
        x9h = xAf[8 * 128:, :].astype(np.float16)
        w1b = np.ascontiguousarray(
            wb.transpose(1, 0, 2, 3).reshape(128, C * CI * CO)).astype(bf)
        w10 = np.ascontiguousarray(
            wb.transpose(1, 2, 0, 3).reshape(128, CI * C * CO)).astype(bf)
        w19 = np.ascontiguousarray(
            wt.reshape(C, NT * CI, CO).transpose(1, 0, 2).reshape(
                128, C * CO)).astype(bf)
        w2b = np.ascontiguousarray(
            wb.transpose(3, 0, 2, 1).reshape(CO, C * CI * 128)).astype(
                np.float16)
        wt = route_weights[:, n0 + NB:n0 + NLOC]
        w29 = np.ascontiguousarray(
            wt.reshape(C, NT * CI, CO).transpose(2, 0, 1).reshape(
                CO, C * 128)).astype(np.float16)
        in_maps.append({"xT3": xT3, "xA": xA, "xT3h": xT3h, "x9h": x9h,
                        "w1b": w1b, "w10": w10, "w19": w19, "w2b": w2b,
                        "w29": w29, "blk": blk})
    return in_maps


_NC_CACHE = {}


def _get_nc():
    if "nc" not in _NC_CACHE:
        _NC_CACHE["nc"] = build_kernel()
    return _NC_CACHE["nc"]


def _postprocess(v: np.ndarray) -> np.ndarray:
    out = v.reshape(CO, C, B).transpose(1, 2, 0)[:, :, None, None, :]
    return np.ascontiguousarray(out.astype(np.float32))


def kernel(x: np.ndarray, route_weights: np.ndarray) -> np.ndarray:
    nc = _get_nc()
    in_maps = _prep_inputs(np.asarray(x, np.float32),
                           np.asarray(route_weights, np.float32))
    res = bass_utils.run_bass_kernel_spmd(nc, in_maps,
                                          core_ids=list(range(NCORES)))
    return _postprocess(np.asarray(res.results[0]["vout"], np.float32))


def kernel_sim(x: np.ndarray, route_weights: np.ndarray) -> np.ndarray:
    """CoreSim (multi-core simulator) path for correctness debugging."""
    from concourse.bass_interp import MultiCoreSim
    nc = _get_nc()
    in_maps = _prep_inputs(np.asarray(x, np.float32),
                           np.asarray(route_weights, np.float32))
    sim = MultiCoreSim(nc, num_cores=NCORES)
    for i, core in sim.cores.items():
        for name, arr in in_maps[i].items():
            core.tensor(name)[:] = arr
    sim.simulate(check_with_hw=False)
    return _postprocess(np.asarray(sim.cores[0].tensor("vout"), np.float32))

